# revision 25
# baseline (speedup 1.0000x reference)
"""Trainium2 Bass kernel for BlockAttnRes.compute_all_inputs (bf16 pipeline).

Math: for each row (b,t), layer l attends over a small per-row source stack.
Sources V = M @ X for a constant 0/1 prefix matrix M (25x25) over the 25 raw
per-row vectors X = [emb, f_0..f_23]. score[l,n] = rsq_n * (v_n . qw_l) with
qw = queries * key_norm_weight, rsq_n = rsqrt(mean(v_n^2)+eps);
h_l = softmax-weighted sum of sources = (A M) @ X.

Device pipeline, super-batches of 4 sub-batches x 5 rows (P=125 = (r,j)):
  1. One 2MB SWDGE (gpsimd) load per super - sprays all 16 SDMA engines,
     single_packet keeps each engine's 128KB chain unfragmented.
  2. Per sub-batch: PE fold-transposes VT_c = X_c.T @ M_bd (prefix fold
     streamed instead of the identity); PSUM->SBUF bf16 evac on ACT/DVE.
     qwT sits in the vt tile gap columns - written once per pool buffer.
  3. PE SC' = VT.T @ [VT | qwT] accumulated over 16 d-chunks
     = [GramV | raw scores]; all stationaries are 128x128 bf16.
  4. DVE eye-masked reduce of GramV diag -> sumsq; ACT rsqrt via exp(-.5 ln);
     softmax smalls quad-merged across the 4 sub-batches ([24,512] tiles).
  5. PE fold alphas through M (BT = M_bd.T @ abd); H = BT.T @ X, 4x N=512;
     PSUM -> SBUF bf16; one contiguous 1.92MB store per super (scalar HWDGE).

Sharding: data-parallel over B*T = 2048 rows -> 8 cores x 256 rows.
bf16 I/O end-to-end: 26MB in + 25MB out per core (rel err ~3e-3 vs 2e-2 gate).
"""

import numpy as np
import ml_dtypes

import concourse.bass as bass
import concourse.bacc as bacc
import concourse.mybir as mybir
from concourse import tile
from concourse.alu_op_type import AluOpType
from concourse.bass_utils import run_bass_kernel_spmd

L = 24
D = 2048
NUM_BLOCKS = 8
EPS = 1e-6
B, T = 2, 1024
N_CORES = 8

ROWS_PER_CORE = (B * T) // N_CORES  # 256
R = 5              # rows per sub-batch
SB = 4             # sub-batches per super-batch
NJ = 25            # raw vectors per row
NS = 25            # sources per row
P = NJ * R         # 125 partitions per sub-batch
NCHUNK = D // 128  # 16 d-chunks
CW = 152           # vt_sb per-chunk pitch: 128 (VT + 3 zero) + 24 qwT
NEG = -1e30

f32 = mybir.dt.float32
bf16 = mybir.dt.bfloat16
BF = ml_dtypes.bfloat16


def _source_matrix():
    M = np.zeros((NS, NJ), dtype=np.float32)
    M[0, 0] = 1.0
    for k in range(NUM_BLOCKS):
        for i in range(3):
            M[1 + 3 * k + i, 1 + 3 * k : 1 + 3 * k + i + 1] = 1.0
    return M


def _valid_matrix():
    V = np.zeros((L, NS), dtype=bool)
    for l in range(L):
        kb, ii = l // 3, l % 3
        V[l, 0] = True
        for k in range(kb):
            V[l, 3 * k + 3] = True
        if ii > 0:
            V[l, 3 * kb + ii] = True
    return V


def _build_consts(queries, key_norm_weight):
    M = _source_matrix()
    valid = _valid_matrix()
    eye_r = np.eye(R, dtype=np.float32)

    qw = (queries * key_norm_weight[None, :]).astype(np.float32)  # [L, D]
    qwT = np.ascontiguousarray(
        qw.reshape(L, NCHUNK, 128).transpose(2, 1, 0).reshape(128, NCHUNK * L)
    ).astype(BF)

    # mtbd[(r,j),(r',n)] = (r==r') * M[n,j]; padded to [128, 128] (zeros)
    mtbd = np.zeros((128, 128), np.float32)
    mtbd[:P, :P] = np.einsum("nj,ab->ajbn", M, eye_r).reshape(P, NS * R)
    mtbd = mtbd.astype(BF)
    # mbd[(r,n),(r',j)] = (r==r') * M[n,j]; padded to [128, 128]
    mbd = np.zeros((128, 128), np.float32)
    mbd[:P, :P] = np.einsum("nj,ab->anbj", M, eye_r).reshape(NS * R, P)
    mbd = mbd.astype(BF)
    # eye mask for GramV diagonal extraction
    eye_bd = np.zeros((P, 128), np.float32)
    eye_bd[:, :P] = np.eye(P, dtype=np.float32)
    # diagm[(r,n),(r',l)] = (r==r')
    diagm = np.einsum("ab,nl->anbl", eye_r, np.ones((NS, L), np.float32))
    diagm = np.ascontiguousarray(diagm.reshape(P, R * L)).astype(np.float32)
    # maskneg[32*s + l, (r,n)] = 0 if valid else NEG; pad rows all NEG
    m1 = np.full((32, P), NEG, np.float32)
    m1[:L, :] = np.where(
        np.broadcast_to(valid[:, None, :], (L, R, NS)).reshape(L, R * NS),
        0.0, NEG)
    maskneg = np.tile(m1, (SB, 1)).astype(np.float32)  # [128, 125]
    ident = np.eye(128, dtype=np.float32)
    return dict(qwT=qwT, mtbd=mtbd, mbd=mbd, eyebd=eye_bd, diagm=diagm,
                maskneg=maskneg, ident=ident)


def _super_starts():
    rows_per_super = R * SB  # 20
    starts = [rows_per_super * b for b in range(ROWS_PER_CORE // rows_per_super)]
    if starts[-1] + rows_per_super < ROWS_PER_CORE:
        starts.append(ROWS_PER_CORE - rows_per_super)  # overlap; idempotent
    return starts


def build_kernel(do_compile=True):
    nc = bacc.Bacc("TRN2", target_bir_lowering=False, debug=False)

    loT = nc.dram_tensor("loT", [ROWS_PER_CORE * NJ, D], bf16,
                         kind="ExternalInput").ap()
    qwT_d = nc.dram_tensor("qwT", [128, NCHUNK * L], bf16, kind="ExternalInput").ap()
    mtbd_d = nc.dram_tensor("mtbd", [128, 128], bf16, kind="ExternalInput").ap()
    mbd_d = nc.dram_tensor("mbd", [128, 128], bf16, kind="ExternalInput").ap()
    eyebd_d = nc.dram_tensor("eyebd", [P, 128], f32, kind="ExternalInput").ap()
    diagm_d = nc.dram_tensor("diagm", [P, R * L], f32, kind="ExternalInput").ap()
    maskneg_d = nc.dram_tensor("maskneg", [128, P], f32,
                               kind="ExternalInput").ap()
    ident_d = nc.dram_tensor("ident", [128, 128], f32, kind="ExternalInput").ap()
    outT = nc.dram_tensor("outT", [ROWS_PER_CORE * L, D], bf16,
                          kind="ExternalOutput").ap()

    with tile.TileContext(nc) as tc:
        with (
            tc.tile_pool(name="const", bufs=1) as const,
            tc.tile_pool(name="xpool", bufs=3) as xpool,
            tc.tile_pool(name="vtpool", bufs=3) as vtpool,
            tc.tile_pool(name="hpool", bufs=2) as hpool,
            tc.tile_pool(name="small", bufs=2) as small,
            tc.tile_pool(name="ps_ft", bufs=2, space=bass.MemorySpace.PSUM) as ps_ft,
            tc.tile_pool(name="ps_sc", bufs=2, space=bass.MemorySpace.PSUM) as ps_sc,
            tc.tile_pool(name="ps_sm", bufs=1, space=bass.MemorySpace.PSUM) as ps_sm,
            tc.tile_pool(name="ps_bt", bufs=1, space=bass.MemorySpace.PSUM) as ps_bt,
            tc.tile_pool(name="ps_h", bufs=2, space=bass.MemorySpace.PSUM) as ps_h,
        ):
            qwT = const.tile([128, NCHUNK * L], bf16)
            nc.sync.dma_start(qwT[:], qwT_d[:])
            mtbd = const.tile([128, 128], bf16)
            nc.sync.dma_start(mtbd[:], mtbd_d[:])
            mbd = const.tile([128, 128], bf16)
            nc.sync.dma_start(mbd[:], mbd_d[:])
            eyebd = const.tile([P, 128], f32)
            nc.sync.dma_start(eyebd[:], eyebd_d[:])
            diagm = const.tile([P, R * L], f32)
            nc.sync.dma_start(diagm[:], diagm_d[:])
            maskneg = const.tile([128, P], f32)
            nc.sync.dma_start(maskneg[:], maskneg_d[:])
            ident = const.tile([128, 128], f32)
            nc.sync.dma_start(ident[:], ident_d[:])
            epsb = const.tile([P, 1], f32)
            nc.vector.memset(epsb[:], EPS)

            s_idx = 0  # global sub-batch counter (vt pool rotation tricks)
            sup_idx = 0
            for row0 in _super_starts():
                # ---- one 2MB SWDGE load: 4 sub-batches, single packet/engine
                # 128-partition tile; rows 125-127 are garbage but inert
                # (every matmul pairs them with zero rows of the constants).
                X4 = xpool.tile([128, SB, D], bf16)
                if sup_idx < 3:  # zero pad partitions once per buffer (the
                    # load then rewrites 96-124; only 125-127 stay zero)
                    nc.vector.memset(X4[96:128], 0.0)
                nc.gpsimd.dma_start(
                    X4[0:P],
                    loT[row0 * NJ : (row0 + R * SB) * NJ, :]
                    .rearrange("(s p) d -> p s d", p=P),
                )

                h4 = hpool.tile([R * L, SB, D], bf16)
                scoreT4 = ps_sm.tile([128, P], f32, tag="sm")
                sumsq4 = small.tile([P, SB], f32, tag="ssq")
                srw4 = small.tile([P, SB * L], f32, tag="srw")
                for s in range(SB):
                    # ---- fold-transposes: VT_c = X_c.T @ M_bd
                    vt_sb = vtpool.tile([128, NCHUNK * CW], bf16)
                    vt3 = vt_sb.rearrange("p (c w) -> p c w", w=CW)
                    if s_idx < 3:  # qwT persists in the 3 rotating buffers
                        nc.scalar.copy(
                            vt3[:, :, 128 : 128 + L],
                            qwT.rearrange("p (c w) -> p c w", w=L),
                        )
                    for half in range(4):
                        ftp = ps_ft.tile([128, 512], f32)
                        for cc in range(4):
                            c = 4 * half + cc
                            nc.tensor.matmul(
                                ftp[:, 128 * cc : 128 * (cc + 1)],
                                X4[:, s, 128 * c : 128 * (c + 1)],
                                mtbd[:],
                                start=True,
                                stop=True,
                            )
                        ft4 = ftp.rearrange("p (cc w) -> p cc w", w=128)
                        dst = vt3[:, 4 * half : 4 * half + 4, 0:128]
                        if half % 2 == 0:
                            nc.scalar.copy(dst, ft4)
                        else:
                            nc.vector.tensor_copy(dst, ft4)

                    # ---- SC' = [GramV | raw scores] over 16 d-chunks
                    SCp = ps_sc.tile([128, CW], f32)
                    for c in range(NCHUNK):
                        nc.tensor.matmul(
                            SCp[:],
                            vt3[:, c, 0:128],
                            vt3[:, c, 0:CW],
                            start=(c == 0),
                            stop=(c == NCHUNK - 1),
                        )
                    # ---- sumsq_n = GramV[n, n]; raw score cols -> SBUF
                    junk = small.tile([P, 128], f32)
                    nc.vector.scalar_tensor_tensor(
                        out=junk[:],
                        in0=SCp[0:P, 0:128],
                        scalar=1.0,
                        in1=eyebd[:],
                        op0=AluOpType.mult,
                        op1=AluOpType.mult,
                        accum_out=sumsq4[:, s : s + 1],
                    )
                    nc.scalar.copy(
                        srw4[:, L * s : L * s + L], SCp[0:P, 128 : 128 + L]
                    )
                    s_idx += 1

                # ---- rsq = exp(-0.5 * ln(sumsq/D + eps)), one op per super
                lnu4 = small.tile([P, SB], f32)
                nc.scalar.activation(
                    lnu4[:], sumsq4[:], mybir.ActivationFunctionType.Ln,
                    bias=epsb[:], scale=1.0 / D,
                )
                rsq4 = small.tile([P, SB], f32)
                nc.scalar.activation(
                    rsq4[:], lnu4[:], mybir.ActivationFunctionType.Exp, scale=-0.5
                )
                # scoresR4: 32-col pitch per sub-batch (cols 24-31 junk)
                scoresR4 = small.tile([P, 32 * SB], f32, tag="srw")
                nc.vector.scalar_tensor_tensor(
                    out=scoresR4.rearrange("p (s w) -> p s w", w=32)[:, :, 0:L],
                    in0=srw4.rearrange("p (s l) -> p s l", l=L),
                    scalar=1.0,
                    in1=rsq4.unsqueeze(2).broadcast_to([P, SB, L]),
                    op0=AluOpType.mult,
                    op1=AluOpType.mult,
                )
                # one transpose into partition-stacked layout [32s + l, (r,n)]
                nc.tensor.transpose(scoreT4[:], scoresR4[:], ident[:P, 0:P])

                # ---- masked softmax, sub-batches stacked on partitions
                smask4 = small.tile([128, P], f32, tag="soft")
                nc.vector.tensor_add(smask4[:], scoreT4[:], maskneg[:])
                esc4 = small.tile([128, P], f32, tag="soft")
                nc.scalar.activation(
                    esc4[:], smask4[:], mybir.ActivationFunctionType.Exp
                )
                ssum4 = small.tile([128, R], f32)
                nc.vector.reduce_sum(
                    ssum4[:],
                    esc4.rearrange("p (r n) -> p r n", n=NS),
                    axis=mybir.AxisListType.X,
                )
                rec4 = small.tile([128, R], f32)
                nc.vector.reciprocal(rec4[:], ssum4[:])
                alpha4 = small.tile([128, P], f32, tag="soft")
                nc.vector.tensor_tensor(
                    alpha4.rearrange("p (r n) -> p r n", n=NS),
                    esc4.rearrange("p (r n) -> p r n", n=NS),
                    rec4.unsqueeze(2).broadcast_to([128, R, NS]),
                    AluOpType.mult,
                )

                # ---- fold alphas through M: BT = M_bd.T @ abd, per sub-batch
                # alphaT4[(r,n), 32s + l]; junk cols 24-31 per 32 never read
                alphaT4 = ps_sm.tile([P, 32 * SB], f32, tag="sm")
                nc.tensor.transpose(alphaT4[:], alpha4[:, 0:P], ident[:, :])
                abd4 = small.tile([128, SB * 128], bf16, tag="abd")
                if sup_idx < 2:  # zero pads persist in the 2 rotating buffers
                    nc.vector.memset(
                        abd4.rearrange("p (s w) -> p s w", w=128)[:, :, 120:128],
                        0.0,
                    )
                    nc.vector.memset(abd4[96:128], 0.0)
                for s in range(SB):
                    nc.vector.scalar_tensor_tensor(
                        out=abd4[0:P, 128 * s : 128 * s + 120]
                        .rearrange("p (r l) -> p r l", l=L),
                        in0=alphaT4[:, 32 * s : 32 * s + L]
                        .unsqueeze(1).broadcast_to([P, R, L]),
                        scalar=1.0,
                        in1=diagm.rearrange("p (r l) -> p r l", l=L),
                        op0=AluOpType.mult,
                        op1=AluOpType.mult,
                    )

                for s in range(SB):
                    BTp = ps_bt.tile([128, 128], f32)
                    nc.tensor.matmul(
                        BTp[:], mbd[:], abd4[:, 128 * s : 128 * (s + 1)],
                        start=True, stop=True,
                    )
                    btsb = small.tile([128, 128], bf16, tag="bt")
                    nc.scalar.copy(btsb[:], BTp[:])

                    # ---- H = BT.T @ X (bf16, N=512 per PSUM bank)
                    for nb in range(4):
                        Hp = ps_h.tile([128, 512], f32)
                        nc.tensor.matmul(
                            Hp[:],
                            btsb[:],
                            X4[:, s, 512 * nb : 512 * (nb + 1)],
                            start=True,
                            stop=True,
                        )
                        if nb % 2 == 0:
                            nc.scalar.copy(
                                h4[:, s, 512 * nb : 512 * (nb + 1)],
                                Hp[0 : R * L, :],
                            )
                        else:
                            nc.vector.tensor_copy(
                                h4[:, s, 512 * nb : 512 * (nb + 1)],
                                Hp[0 : R * L, :],
                            )

                # ---- one contiguous 1.92MB store per super (ACT HWDGE ring)
                nc.scalar.dma_start(
                    outT[row0 * L : (row0 + R * SB) * L, :]
                    .rearrange("(s p) d -> p s d", p=R * L),
                    h4[:],
                )
                sup_idx += 1

    real_gat = bacc.get_activation_tables
    AF = mybir.ActivationFunctionType

    def gat_pinned(arch):
        out = {}
        for name, fns in real_gat(arch).items():
            if name == "natural_log_exp_and_others":
                out[name] = set(fns)
            else:
                out[name] = {f for f in fns if f not in (AF.Ln, AF.Exp)}
        return out

    bacc.get_activation_tables = gat_pinned
    try:
        if do_compile:
            nc.compile()
    finally:
        bacc.get_activation_tables = real_gat
    return nc


_NC_CACHE = None


def _prep_loT(layer_outputs, embedding):
    loT = np.empty((B * T, NJ, D), dtype=BF)
    loT[:, 0, :] = embedding.reshape(B * T, D).astype(BF)
    loT[:, 1:, :] = (
        layer_outputs.reshape(L, B * T, D).transpose(1, 0, 2).astype(BF)
    )
    return loT


def _make_in_maps(layer_outputs, embedding, queries, key_norm_weight):
    loT = _prep_loT(layer_outputs, embedding)
    consts = _build_consts(queries, key_norm_weight)
    in_maps = []
    for c in range(N_CORES):
        r0 = c * ROWS_PER_CORE
        in_maps.append({
            "loT": loT[r0 : r0 + ROWS_PER_CORE].reshape(ROWS_PER_CORE * NJ, D),
            "qwT": consts["qwT"],
            "mtbd": consts["mtbd"],
            "mbd": consts["mbd"],
            "eyebd": consts["eyebd"],
            "diagm": consts["diagm"],
            "maskneg": consts["maskneg"],
            "ident": consts["ident"],
        })
    return in_maps


def kernel(layer_outputs, embedding, queries, key_norm_weight):
    global _NC_CACHE
    layer_outputs = np.asarray(layer_outputs, dtype=np.float32)
    embedding = np.asarray(embedding, dtype=np.float32)
    queries = np.asarray(queries, dtype=np.float32)
    key_norm_weight = np.asarray(key_norm_weight, dtype=np.float32)

    in_maps = _make_in_maps(layer_outputs, embedding, queries, key_norm_weight)

    if _NC_CACHE is None:
        _NC_CACHE = build_kernel()
    nc = _NC_CACHE

    res = run_bass_kernel_spmd(nc, in_maps, core_ids=list(range(N_CORES)))

    full = np.empty((L, B * T, D), dtype=np.float32)
    for c in range(N_CORES):
        r0 = c * ROWS_PER_CORE
        outT = res.results[c]["outT"].astype(np.float32).reshape(
            ROWS_PER_CORE, L, D
        )
        full[:, r0 : r0 + ROWS_PER_CORE, :] = outT.transpose(1, 0, 2)
    return full.reshape(L, B, T, D)


# revision 26
# speedup vs baseline: 1.0464x; 1.0464x over previous
"""Trainium2 Bass kernel for BlockAttnRes.compute_all_inputs (bf16 pipeline).

Math: for each row (b,t), layer l attends over a small per-row source stack.
Sources V = M @ X for a constant 0/1 prefix matrix M (25x25) over the 25 raw
per-row vectors X = [emb, f_0..f_23]. score[l,n] = rsq_n * (v_n . qw_l) with
qw = queries * key_norm_weight, rsq_n = rsqrt(mean(v_n^2)+eps);
h_l = softmax-weighted sum of sources = (A M) @ X.

Device pipeline, super-batches of 4 sub-batches x 5 rows (P=125 = (r,j)):
  1. One 2MB SWDGE (gpsimd) load per super. Host stores the input p-major
     per super ([125, 4 subs, 2048]) so every partition's slice is one
     16KB contiguous HBM run -> long per-engine descriptor chains across
     all 16 SDMA engines. Store side mirrors this ([120, 4, 2048], 16KB).
  2. Per sub-batch: 16 pure bf16 PE transposes (bf16 PSUM -> 2x-rate DVE/ACT
     evac); qwT parked in the vt tile gap columns (written once per buffer).
  3. PE SC = XT.T @ [XT | qwT] over 16 d-chunks = [GramX | raw q-dots];
     one M-fold matmul (mtbd.T @ SC) -> [v_n.x_j' | v_n.qw_l].
  4. DVE mbd-masked reduce -> sumsq; ACT rsqrt via exp(-.5 ln); softmax
     smalls merged across the 4 sub-batches (partition-stacked [128, 125]).
  5. PE fold alphas through M (BT = M_bd.T @ abd); H = BT.T @ X, 4x N=512;
     PSUM -> SBUF bf16; one contiguous 1.92MB store per super (scalar HWDGE).

Sharding: data-parallel over B*T = 2048 rows -> 8 cores x 256 rows.
bf16 I/O end-to-end: 26MB in + 25MB out per core (rel err ~3e-3 vs 2e-2 gate).
"""

import numpy as np
import ml_dtypes

import concourse.bass as bass
import concourse.bacc as bacc
import concourse.mybir as mybir
from concourse import tile
from concourse.alu_op_type import AluOpType
from concourse.bass_utils import run_bass_kernel_spmd

L = 24
D = 2048
NUM_BLOCKS = 8
EPS = 1e-6
B, T = 2, 1024
N_CORES = 8

ROWS_PER_CORE = (B * T) // N_CORES  # 256
R = 5              # rows per sub-batch
SB = 4             # sub-batches per super-batch
NJ = 25            # raw vectors per row
NS = 25            # sources per row
P = NJ * R         # 125 partitions per sub-batch
NCHUNK = D // 128  # 16 d-chunks
CW = 152           # vt_sb per-chunk pitch: 128 (XT + 3 zero) + 24 qwT
NEG = -1e30

f32 = mybir.dt.float32
bf16 = mybir.dt.bfloat16
BF = ml_dtypes.bfloat16


def _source_matrix():
    M = np.zeros((NS, NJ), dtype=np.float32)
    M[0, 0] = 1.0
    for k in range(NUM_BLOCKS):
        for i in range(3):
            M[1 + 3 * k + i, 1 + 3 * k : 1 + 3 * k + i + 1] = 1.0
    return M


def _valid_matrix():
    V = np.zeros((L, NS), dtype=bool)
    for l in range(L):
        kb, ii = l // 3, l % 3
        V[l, 0] = True
        for k in range(kb):
            V[l, 3 * k + 3] = True
        if ii > 0:
            V[l, 3 * kb + ii] = True
    return V


def _build_consts(queries, key_norm_weight):
    M = _source_matrix()
    valid = _valid_matrix()
    eye_r = np.eye(R, dtype=np.float32)

    qw = (queries * key_norm_weight[None, :]).astype(np.float32)  # [L, D]
    qwT = np.ascontiguousarray(
        qw.reshape(L, NCHUNK, 128).transpose(2, 1, 0).reshape(128, NCHUNK * L)
    ).astype(BF)

    # mtbd[(r,j),(r',n)] = (r==r') * M[n,j]; padded to [128, 128] (zeros)
    mtbd = np.zeros((128, 128), np.float32)
    mtbd[:P, :P] = np.einsum("nj,ab->ajbn", M, eye_r).reshape(P, NS * R)
    mtbd = mtbd.astype(BF)
    # mbd[(r,n),(r',j)] = (r==r') * M[n,j]; padded to [128, 128]
    mbd128 = np.zeros((128, 128), np.float32)
    mbd128[:P, :P] = np.einsum("nj,ab->anbj", M, eye_r).reshape(NS * R, P)
    mbd = mbd128.astype(BF)
    mbdf = mbd128[:P].astype(np.float32)  # [125, 128] f32 sumsq mask
    # diagm[(r,n),(r',l)] = (r==r')
    diagm = np.einsum("ab,nl->anbl", eye_r, np.ones((NS, L), np.float32))
    diagm = np.ascontiguousarray(diagm.reshape(P, R * L)).astype(np.float32)
    # maskneg[32*s + l, (r,n)] = 0 if valid else NEG; pad rows all NEG
    m1 = np.full((32, P), NEG, np.float32)
    m1[:L, :] = np.where(
        np.broadcast_to(valid[:, None, :], (L, R, NS)).reshape(L, R * NS),
        0.0, NEG)
    maskneg = np.tile(m1, (SB, 1)).astype(np.float32)  # [128, 125]
    ident = np.eye(128, dtype=np.float32)
    identb = np.eye(128, dtype=np.float32).astype(BF)
    return dict(qwT=qwT, mtbd=mtbd, mbd=mbd, mbdf=mbdf, diagm=diagm,
                maskneg=maskneg, ident=ident, identb=identb)


def _super_starts():
    rows_per_super = R * SB  # 20
    starts = [rows_per_super * b for b in range(ROWS_PER_CORE // rows_per_super)]
    if starts[-1] + rows_per_super < ROWS_PER_CORE:
        starts.append(ROWS_PER_CORE - rows_per_super)  # overlap; idempotent
    return starts


NSUP = len(_super_starts())  # 13


def build_kernel(do_compile=True):
    nc = bacc.Bacc("TRN2", target_bir_lowering=False, debug=False)

    # p-major per super: row sup*125 + p holds that partition's 4 sub-batch
    # rows back to back -> 16KB contiguous per partition per load.
    loT = nc.dram_tensor("loT", [NSUP * P, SB * D], bf16,
                         kind="ExternalInput").ap()
    qwT_d = nc.dram_tensor("qwT", [128, NCHUNK * L], bf16, kind="ExternalInput").ap()
    mtbd_d = nc.dram_tensor("mtbd", [128, 128], bf16, kind="ExternalInput").ap()
    mbd_d = nc.dram_tensor("mbd", [128, 128], bf16, kind="ExternalInput").ap()
    mbdf_d = nc.dram_tensor("mbdf", [P, 128], f32, kind="ExternalInput").ap()
    diagm_d = nc.dram_tensor("diagm", [P, R * L], f32, kind="ExternalInput").ap()
    maskneg_d = nc.dram_tensor("maskneg", [128, P], f32,
                               kind="ExternalInput").ap()
    ident_d = nc.dram_tensor("ident", [128, 128], f32, kind="ExternalInput").ap()
    identb_d = nc.dram_tensor("identb", [128, 128], bf16, kind="ExternalInput").ap()
    # p-major output: row sup*120 + p holds (s, d) -> 16KB contiguous
    outT = nc.dram_tensor("outT", [NSUP * R * L, SB * D], bf16,
                          kind="ExternalOutput").ap()

    with tile.TileContext(nc) as tc:
        with (
            tc.tile_pool(name="const", bufs=1) as const,
            tc.tile_pool(name="xpool", bufs=3) as xpool,
            tc.tile_pool(name="vtpool", bufs=3) as vtpool,
            tc.tile_pool(name="hpool", bufs=2) as hpool,
            tc.tile_pool(name="small", bufs=2) as small,
            tc.tile_pool(name="ps_ft", bufs=2, space=bass.MemorySpace.PSUM) as ps_ft,
            tc.tile_pool(name="ps_sc", bufs=1, space=bass.MemorySpace.PSUM) as ps_sc,
            tc.tile_pool(name="ps_mo", bufs=1, space=bass.MemorySpace.PSUM) as ps_mo,
            tc.tile_pool(name="ps_sm", bufs=1, space=bass.MemorySpace.PSUM) as ps_sm,
            tc.tile_pool(name="ps_bt", bufs=1, space=bass.MemorySpace.PSUM) as ps_bt,
            tc.tile_pool(name="ps_h", bufs=2, space=bass.MemorySpace.PSUM) as ps_h,
        ):
            qwT = const.tile([128, NCHUNK * L], bf16)
            nc.sync.dma_start(qwT[:], qwT_d[:])
            mtbd = const.tile([128, 128], bf16)
            nc.sync.dma_start(mtbd[:], mtbd_d[:])
            mbd = const.tile([128, 128], bf16)
            nc.sync.dma_start(mbd[:], mbd_d[:])
            mbdf = const.tile([P, 128], f32)
            nc.sync.dma_start(mbdf[:], mbdf_d[:])
            diagm = const.tile([P, R * L], f32)
            nc.sync.dma_start(diagm[:], diagm_d[:])
            maskneg = const.tile([128, P], f32)
            nc.sync.dma_start(maskneg[:], maskneg_d[:])
            ident = const.tile([128, 128], f32)
            nc.sync.dma_start(ident[:], ident_d[:])
            identb = const.tile([128, 128], bf16)
            nc.sync.dma_start(identb[:], identb_d[:])
            epsb = const.tile([P, 1], f32)
            nc.vector.memset(epsb[:], EPS)

            s_idx = 0  # global sub-batch counter (vt pool rotation tricks)
            for sup_idx in range(NSUP):
                # ---- one 2MB SWDGE load (16KB contiguous per partition)
                X4 = xpool.tile([128, SB, D], bf16)
                if sup_idx < 3:  # zero pad partitions once per buffer (the
                    # load then rewrites 96-124; only 125-127 stay zero)
                    nc.vector.memset(X4[96:128], 0.0)
                nc.gpsimd.dma_start(
                    X4[0:P],
                    loT[sup_idx * P : (sup_idx + 1) * P, :]
                    .rearrange("p (s d) -> p s d", d=D),
                )

                h4 = hpool.tile([R * L, SB, D], bf16)
                scoreT4 = ps_sm.tile([128, P], f32, tag="sm")
                sumsq4 = small.tile([P, SB], f32, tag="ssq")
                srw4 = small.tile([P, SB * L], f32, tag="srw")
                for s in range(SB):
                    # ---- pure bf16 transposes: XT_c = X_c.T (bf16 PSUM)
                    vt_sb = vtpool.tile([128, NCHUNK * CW], bf16)
                    vt3 = vt_sb.rearrange("p (c w) -> p c w", w=CW)
                    if s_idx < 3:  # qwT persists in the 3 rotating buffers
                        nc.scalar.copy(
                            vt3[:, :, 128 : 128 + L],
                            qwT.rearrange("p (c w) -> p c w", w=L),
                        )
                    for half in range(2):
                        ftp = ps_ft.tile([128, 1024], bf16)
                        for cc in range(8):
                            c = 8 * half + cc
                            nc.tensor.transpose(
                                ftp[:, 128 * cc : 128 * (cc + 1)],
                                X4[:, s, 128 * c : 128 * (c + 1)],
                                identb[:],
                            )
                        ft8 = ftp.rearrange("p (cc w) -> p cc w", w=128)
                        dst = vt3[:, 8 * half : 8 * half + 8, 0:128]
                        if half == 0:
                            nc.scalar.copy(dst, ft8)
                        else:
                            nc.vector.tensor_copy(dst, ft8)

                    # ---- SC = [GramX | raw q-dots] over 16 d-chunks
                    SCp = ps_sc.tile([128, CW], f32)
                    for c in range(NCHUNK):
                        nc.tensor.matmul(
                            SCp[:],
                            vt3[:, c, 0:128],
                            vt3[:, c, 0:CW],
                            start=(c == 0),
                            stop=(c == NCHUNK - 1),
                        )
                    SC_sb = small.tile([128, CW], bf16, tag="scsb")
                    nc.scalar.copy(SC_sb[:], SCp[:])

                    # ---- M-fold: Mout = [v_n.x_j' | v_n.qw_l]
                    Mout = ps_mo.tile([128, CW], f32)
                    nc.tensor.matmul(
                        Mout[:], mtbd[:], SC_sb[:], start=True, stop=True
                    )

                    # ---- sumsq via mbd-masked row reduce; raw scores -> SBUF
                    junk = small.tile([P, 128], f32)
                    nc.vector.scalar_tensor_tensor(
                        out=junk[:],
                        in0=Mout[0:P, 0:128],
                        scalar=1.0,
                        in1=mbdf[:],
                        op0=AluOpType.mult,
                        op1=AluOpType.mult,
                        accum_out=sumsq4[:, s : s + 1],
                    )
                    nc.scalar.copy(
                        srw4[:, L * s : L * s + L], Mout[0:P, 128 : 128 + L]
                    )
                    s_idx += 1

                # ---- rsq = exp(-0.5 * ln(sumsq/D + eps)), one op per super
                lnu4 = small.tile([P, SB], f32)
                nc.scalar.activation(
                    lnu4[:], sumsq4[:], mybir.ActivationFunctionType.Ln,
                    bias=epsb[:], scale=1.0 / D,
                )
                rsq4 = small.tile([P, SB], f32)
                nc.scalar.activation(
                    rsq4[:], lnu4[:], mybir.ActivationFunctionType.Exp, scale=-0.5
                )
                # scoresR4: 32-col pitch per sub-batch (cols 24-31 junk)
                scoresR4 = small.tile([P, 32 * SB], f32, tag="srw")
                nc.vector.scalar_tensor_tensor(
                    out=scoresR4.rearrange("p (s w) -> p s w", w=32)[:, :, 0:L],
                    in0=srw4.rearrange("p (s l) -> p s l", l=L),
                    scalar=1.0,
                    in1=rsq4.unsqueeze(2).broadcast_to([P, SB, L]),
                    op0=AluOpType.mult,
                    op1=AluOpType.mult,
                )
                # one transpose into partition-stacked layout [32s + l, (r,n)]
                nc.tensor.transpose(scoreT4[:], scoresR4[:], ident[:P, 0:P])

                # ---- masked softmax, sub-batches stacked on partitions
                smask4 = small.tile([128, P], f32, tag="soft")
                nc.vector.tensor_add(smask4[:], scoreT4[:], maskneg[:])
                esc4 = small.tile([128, P], f32, tag="soft")
                nc.scalar.activation(
                    esc4[:], smask4[:], mybir.ActivationFunctionType.Exp
                )
                ssum4 = small.tile([128, R], f32)
                nc.vector.reduce_sum(
                    ssum4[:],
                    esc4.rearrange("p (r n) -> p r n", n=NS),
                    axis=mybir.AxisListType.X,
                )
                rec4 = small.tile([128, R], f32)
                nc.vector.reciprocal(rec4[:], ssum4[:])
                alpha4 = small.tile([128, P], f32, tag="soft")
                nc.vector.tensor_tensor(
                    alpha4.rearrange("p (r n) -> p r n", n=NS),
                    esc4.rearrange("p (r n) -> p r n", n=NS),
                    rec4.unsqueeze(2).broadcast_to([128, R, NS]),
                    AluOpType.mult,
                )

                # ---- fold alphas through M: BT = M_bd.T @ abd, per sub-batch
                # alphaT4[(r,n), 32s + l]; junk cols 24-31 per 32 never read
                alphaT4 = ps_sm.tile([P, 32 * SB], f32, tag="sm")
                nc.tensor.transpose(alphaT4[:], alpha4[:, 0:P], ident[:, :])
                abd4 = small.tile([128, SB * 128], bf16, tag="abd")
                if sup_idx < 2:  # zero pads persist in the 2 rotating buffers
                    nc.vector.memset(
                        abd4.rearrange("p (s w) -> p s w", w=128)[:, :, 120:128],
                        0.0,
                    )
                    nc.vector.memset(abd4[96:128], 0.0)
                for s in range(SB):
                    nc.vector.scalar_tensor_tensor(
                        out=abd4[0:P, 128 * s : 128 * s + 120]
                        .rearrange("p (r l) -> p r l", l=L),
                        in0=alphaT4[:, 32 * s : 32 * s + L]
                        .unsqueeze(1).broadcast_to([P, R, L]),
                        scalar=1.0,
                        in1=diagm.rearrange("p (r l) -> p r l", l=L),
                        op0=AluOpType.mult,
                        op1=AluOpType.mult,
                    )

                for s in range(SB):
                    BTp = ps_bt.tile([128, 128], f32)
                    nc.tensor.matmul(
                        BTp[:], mbd[:], abd4[:, 128 * s : 128 * (s + 1)],
                        start=True, stop=True,
                    )
                    btsb = small.tile([128, 128], bf16, tag="bt")
                    nc.scalar.copy(btsb[:], BTp[:])

                    # ---- H = BT.T @ X (bf16, N=512 per PSUM bank)
                    for nb in range(4):
                        Hp = ps_h.tile([128, 512], f32)
                        nc.tensor.matmul(
                            Hp[:],
                            btsb[:],
                            X4[:, s, 512 * nb : 512 * (nb + 1)],
                            start=True,
                            stop=True,
                        )
                        if nb % 2 == 0:
                            nc.scalar.copy(
                                h4[:, s, 512 * nb : 512 * (nb + 1)],
                                Hp[0 : R * L, :],
                            )
                        else:
                            nc.vector.tensor_copy(
                                h4[:, s, 512 * nb : 512 * (nb + 1)],
                                Hp[0 : R * L, :],
                            )

                # ---- one contiguous 1.92MB store per super (ACT HWDGE ring)
                nc.scalar.dma_start(
                    outT[sup_idx * R * L : (sup_idx + 1) * R * L, :]
                    .rearrange("p (s d) -> p s d", d=D),
                    h4[:],
                )

    real_gat = bacc.get_activation_tables
    AF = mybir.ActivationFunctionType

    def gat_pinned(arch):
        out = {}
        for name, fns in real_gat(arch).items():
            if name == "natural_log_exp_and_others":
                out[name] = set(fns)
            else:
                out[name] = {f for f in fns if f not in (AF.Ln, AF.Exp)}
        return out

    bacc.get_activation_tables = gat_pinned
    try:
        if do_compile:
            nc.compile()
    finally:
        bacc.get_activation_tables = real_gat
    return nc


_NC_CACHE = None


def _prep_loT(layer_outputs, embedding):
    """[L,B,T,D]+[B,T,D] -> p-major per-core per-super stacks
    [N_CORES, NSUP * 125, SB * D] bf16."""
    lo3 = np.empty((B * T, NJ, D), dtype=BF)
    lo3[:, 0, :] = embedding.reshape(B * T, D).astype(BF)
    lo3[:, 1:, :] = (
        layer_outputs.reshape(L, B * T, D).transpose(1, 0, 2).astype(BF)
    )
    sup = np.asarray(_super_starts())  # [NSUP]
    rows = sup[:, None] + np.arange(R * SB)[None, :]  # [NSUP, 20]
    out = np.empty((N_CORES, NSUP * P, SB * D), dtype=BF)
    for c in range(N_CORES):
        g = lo3[c * ROWS_PER_CORE + rows]          # [NSUP, 20, 25, D]
        g = g.reshape(NSUP, SB, P, D).transpose(0, 2, 1, 3)  # [NSUP,125,SB,D]
        out[c] = g.reshape(NSUP * P, SB * D)
    return out


def _make_in_maps(layer_outputs, embedding, queries, key_norm_weight):
    loT = _prep_loT(layer_outputs, embedding)
    consts = _build_consts(queries, key_norm_weight)
    in_maps = []
    for c in range(N_CORES):
        in_maps.append({
            "loT": loT[c],
            "qwT": consts["qwT"],
            "mtbd": consts["mtbd"],
            "mbd": consts["mbd"],
            "mbdf": consts["mbdf"],
            "diagm": consts["diagm"],
            "maskneg": consts["maskneg"],
            "ident": consts["ident"],
            "identb": consts["identb"],
        })
    return in_maps


def kernel(layer_outputs, embedding, queries, key_norm_weight):
    global _NC_CACHE
    layer_outputs = np.asarray(layer_outputs, dtype=np.float32)
    embedding = np.asarray(embedding, dtype=np.float32)
    queries = np.asarray(queries, dtype=np.float32)
    key_norm_weight = np.asarray(key_norm_weight, dtype=np.float32)

    in_maps = _make_in_maps(layer_outputs, embedding, queries, key_norm_weight)

    if _NC_CACHE is None:
        _NC_CACHE = build_kernel()
    nc = _NC_CACHE

    res = run_bass_kernel_spmd(nc, in_maps, core_ids=list(range(N_CORES)))

    sup = np.asarray(_super_starts())
    rows = (sup[:, None, None] + R * np.arange(SB)[None, :, None]
            + np.arange(R)[None, None, :])  # [NSUP, SB, R]
    full = np.empty((L, B * T, D), dtype=np.float32)
    for c in range(N_CORES):
        o = res.results[c]["outT"].astype(np.float32)
        o = o.reshape(NSUP, R * L, SB, D).transpose(0, 2, 1, 3)
        o = o.reshape(NSUP, SB, R, L, D).transpose(3, 0, 1, 2, 4)
        full[:, c * ROWS_PER_CORE + rows.ravel(), :] = o.reshape(
            L, NSUP * SB * R, D)
    return full.reshape(L, B, T, D)


# revision 30
# speedup vs baseline: 1.2446x; 1.1894x over previous
"""Trainium2 Bass kernel for BlockAttnRes.compute_all_inputs (bf16 pipeline).

Math: for each row (b,t), layer l attends over a small per-row source stack.
Sources V = M @ X for a constant 0/1 prefix matrix M (25x25) over the 25 raw
per-row vectors X = [emb, f_0..f_23]. score[l,n] = rsq_n * (v_n . qw_l) with
qw = queries * key_norm_weight, rsq_n = rsqrt(mean(v_n^2)+eps);
h_l = softmax-weighted sum of sources = (A M) @ X.

Device pipeline, super-batches of 4 sub-batches x 5 rows (P=125 = (r,j)):
  1. One 2MB SWDGE (gpsimd) load per super. Host stores the input p-major
     per super ([125, 4 subs, 2048]) so every partition's slice is one
     16KB contiguous HBM run -> long per-engine descriptor chains across
     all 16 SDMA engines. Store side mirrors this ([120, 4, 2048], 16KB).
  2. Per sub-batch: 16 pure bf16 PE transposes (bf16 PSUM -> 2x-rate DVE/ACT
     evac); qwT parked in the vt tile gap columns (written once per buffer).
  3. PE SC = XT.T @ [XT | qwT] over 16 d-chunks = [GramX | raw q-dots];
     one M-fold matmul (mtbd.T @ SC) -> [v_n.x_j' | v_n.qw_l].
  4. DVE mbd-masked reduce -> sumsq; ACT rsqrt via exp(-.5 ln); softmax
     smalls merged across the 4 sub-batches (partition-stacked [128, 125]).
  5. PE fold alphas through M (BT = M_bd.T @ abd); H = BT.T @ X, 4x N=512;
     PSUM -> SBUF bf16; one contiguous 1.92MB store per super (scalar HWDGE).

Sharding: data-parallel over B*T = 2048 rows -> 8 cores x 256 rows.
bf16 I/O end-to-end: 26MB in + 25MB out per core (rel err ~3e-3 vs 2e-2 gate).
"""

import numpy as np
import ml_dtypes

import concourse.bass as bass
import concourse.bacc as bacc
import concourse.mybir as mybir
from concourse import tile
from concourse.alu_op_type import AluOpType
from concourse.bass_utils import run_bass_kernel_spmd

L = 24
D = 2048
NUM_BLOCKS = 8
EPS = 1e-6
B, T = 2, 1024
N_CORES = 8

ROWS_PER_CORE = (B * T) // N_CORES  # 256
R = 5              # rows per sub-batch
SB = 4             # sub-batches per super-batch
NJ = 25            # raw vectors per row
NS = 25            # sources per row
P = NJ * R         # 125 partitions per sub-batch
NCHUNK = D // 128  # 16 d-chunks
CW = 152           # vt_sb per-chunk pitch: 128 (XT + 3 zero) + 24 qwT
NEG = -1e30

f32 = mybir.dt.float32
bf16 = mybir.dt.bfloat16
BF = ml_dtypes.bfloat16


def _source_matrix():
    M = np.zeros((NS, NJ), dtype=np.float32)
    M[0, 0] = 1.0
    for k in range(NUM_BLOCKS):
        for i in range(3):
            M[1 + 3 * k + i, 1 + 3 * k : 1 + 3 * k + i + 1] = 1.0
    return M


def _valid_matrix():
    V = np.zeros((L, NS), dtype=bool)
    for l in range(L):
        kb, ii = l // 3, l % 3
        V[l, 0] = True
        for k in range(kb):
            V[l, 3 * k + 3] = True
        if ii > 0:
            V[l, 3 * kb + ii] = True
    return V


def _build_consts(queries, key_norm_weight):
    M = _source_matrix()
    valid = _valid_matrix()
    eye_r = np.eye(R, dtype=np.float32)

    qw = (queries * key_norm_weight[None, :]).astype(np.float32)  # [L, D]
    qwT = np.ascontiguousarray(
        qw.reshape(L, NCHUNK, 128).transpose(2, 1, 0).reshape(128, NCHUNK * L)
    ).astype(BF)

    # mtbd[(r,j),(r',n)] = (r==r') * M[n,j]; padded to [128, 128] (zeros)
    mtbd = np.zeros((128, 128), np.float32)
    mtbd[:P, :P] = np.einsum("nj,ab->ajbn", M, eye_r).reshape(P, NS * R)
    mtbd = mtbd.astype(BF)
    # mbd[(r,n),(r',j)] = (r==r') * M[n,j]; padded to [128, 128]
    mbd128 = np.zeros((128, 128), np.float32)
    mbd128[:P, :P] = np.einsum("nj,ab->anbj", M, eye_r).reshape(NS * R, P)
    mbd = mbd128.astype(BF)
    mbdf = mbd128[:P].astype(np.float32)  # [125, 128] f32 sumsq mask
    # diagm[(r,n),(r',l)] = (r==r')
    diagm = np.einsum("ab,nl->anbl", eye_r, np.ones((NS, L), np.float32))
    diagm = np.ascontiguousarray(diagm.reshape(P, R * L)).astype(np.float32)
    # maskneg[l, (r,n)] = 0 if valid else NEG
    maskneg = np.where(
        np.broadcast_to(valid[:, None, :], (L, R, NS)).reshape(L, R * NS),
        0.0, NEG).astype(np.float32)  # [24, 125]
    ident = np.eye(128, dtype=np.float32)
    identb = np.eye(128, dtype=np.float32).astype(BF)
    return dict(qwT=qwT, mtbd=mtbd, mbd=mbd, mbdf=mbdf, diagm=diagm,
                maskneg=maskneg, ident=ident, identb=identb)


def _super_starts():
    rows_per_super = R * SB  # 20
    starts = [rows_per_super * b for b in range(ROWS_PER_CORE // rows_per_super)]
    if starts[-1] + rows_per_super < ROWS_PER_CORE:
        starts.append(ROWS_PER_CORE - rows_per_super)  # overlap; idempotent
    return starts


NSUP = len(_super_starts())  # 13


def build_kernel(do_compile=True):
    nc = bacc.Bacc("TRN2", target_bir_lowering=False, debug=False)

    # p-major per super: row sup*125 + p holds that partition's 4 sub-batch
    # rows back to back -> 16KB contiguous per partition per load.
    loT = nc.dram_tensor("loT", [NSUP * P, SB * D], bf16,
                         kind="ExternalInput").ap()
    qwT_d = nc.dram_tensor("qwT", [128, NCHUNK * L], bf16, kind="ExternalInput").ap()
    mtbd_d = nc.dram_tensor("mtbd", [128, 128], bf16, kind="ExternalInput").ap()
    mbd_d = nc.dram_tensor("mbd", [128, 128], bf16, kind="ExternalInput").ap()
    mbdf_d = nc.dram_tensor("mbdf", [P, 128], f32, kind="ExternalInput").ap()
    diagm_d = nc.dram_tensor("diagm", [P, R * L], f32, kind="ExternalInput").ap()
    maskneg_d = nc.dram_tensor("maskneg", [L, P], f32,
                               kind="ExternalInput").ap()
    ident_d = nc.dram_tensor("ident", [128, 128], f32, kind="ExternalInput").ap()
    identb_d = nc.dram_tensor("identb", [128, 128], bf16, kind="ExternalInput").ap()
    # p-major output: row sup*120 + p holds (s, d) -> 16KB contiguous
    outT = nc.dram_tensor("outT", [NSUP * R * L, SB * D], bf16,
                          kind="ExternalOutput").ap()

    with tile.TileContext(nc) as tc:
        with (
            tc.tile_pool(name="const", bufs=1) as const,
            tc.tile_pool(name="xpool", bufs=3) as xpool,
            tc.tile_pool(name="vtpool", bufs=3) as vtpool,
            tc.tile_pool(name="hpool", bufs=2) as hpool,
            tc.tile_pool(name="small", bufs=2) as small,
            tc.tile_pool(name="ps_ft", bufs=2, space=bass.MemorySpace.PSUM) as ps_ft,
            tc.tile_pool(name="ps_sc", bufs=2, space=bass.MemorySpace.PSUM) as ps_sc,
            tc.tile_pool(name="ps_sm", bufs=2, space=bass.MemorySpace.PSUM) as ps_sm,
            tc.tile_pool(name="ps_h", bufs=2, space=bass.MemorySpace.PSUM) as ps_h,
        ):
            qwT = const.tile([128, NCHUNK * L], bf16)
            nc.sync.dma_start(qwT[:], qwT_d[:])
            mtbd = const.tile([128, 128], bf16)
            nc.sync.dma_start(mtbd[:], mtbd_d[:])
            mbd = const.tile([128, 128], bf16)
            nc.sync.dma_start(mbd[:], mbd_d[:])
            mbdf = const.tile([P, 128], f32)
            nc.sync.dma_start(mbdf[:], mbdf_d[:])
            diagm = const.tile([P, R * L], f32)
            nc.sync.dma_start(diagm[:], diagm_d[:])
            maskneg = const.tile([L, P], f32)
            nc.sync.dma_start(maskneg[:], maskneg_d[:])
            ident = const.tile([128, 128], f32)
            nc.sync.dma_start(ident[:], ident_d[:])
            identb = const.tile([128, 128], bf16)
            nc.sync.dma_start(identb[:], identb_d[:])
            epsb = const.tile([P, 1], f32)
            nc.vector.memset(epsb[:], EPS)

            s_idx = 0  # global sub-batch counter (vt pool rotation tricks)
            for sup_idx in range(NSUP):
                # ---- one 2MB SWDGE load (16KB contiguous per partition)
                X4 = xpool.tile([128, SB, D], bf16)
                if sup_idx < 3:  # zero pad partitions once per buffer (the
                    # load then rewrites 96-124; only 125-127 stay zero)
                    nc.vector.memset(X4[96:128], 0.0)
                nc.gpsimd.dma_start(
                    X4[0:P],
                    loT[sup_idx * P : (sup_idx + 1) * P, :]
                    .rearrange("p (s d) -> p s d", d=D),
                )

                h4 = hpool.tile([R * L, SB, D], bf16)
                for s in range(SB):
                    # ---- pure bf16 transposes: XT_c = X_c.T (bf16 PSUM)
                    vt_sb = vtpool.tile([128, NCHUNK * CW], bf16)
                    vt3 = vt_sb.rearrange("p (c w) -> p c w", w=CW)
                    if s_idx < 3:  # qwT persists in the 3 rotating buffers
                        nc.scalar.copy(
                            vt3[:, :, 128 : 128 + L],
                            qwT.rearrange("p (c w) -> p c w", w=L),
                        )
                    for half in range(4):
                        ftp = ps_ft.tile([128, 512], bf16)
                        for cc in range(4):
                            c = 4 * half + cc
                            nc.tensor.transpose(
                                ftp[:, 128 * cc : 128 * (cc + 1)],
                                X4[:, s, 128 * c : 128 * (c + 1)],
                                identb[:],
                            )
                        ft4 = ftp.rearrange("p (cc w) -> p cc w", w=128)
                        dst = vt3[:, 4 * half : 4 * half + 4, 0:128]
                        if half % 2 == 0:
                            nc.scalar.copy(dst, ft4)
                        else:
                            nc.vector.tensor_copy(dst, ft4)

                    # ---- SC = [GramX | raw q-dots] over 16 d-chunks
                    SCp = ps_sc.tile([128, CW], f32, tag="sc")
                    for c in range(NCHUNK):
                        nc.tensor.matmul(
                            SCp[:],
                            vt3[:, c, 0:128],
                            vt3[:, c, 0:CW],
                            start=(c == 0),
                            stop=(c == NCHUNK - 1),
                        )
                    SC_sb = small.tile([128, CW], bf16, tag="scsb")
                    nc.scalar.copy(SC_sb[:], SCp[:])

                    # ---- M-fold: Mout = [v_n.x_j' | v_n.qw_l]
                    Mout = ps_sc.tile([128, CW], f32, tag="sc")
                    nc.tensor.matmul(
                        Mout[:], mtbd[:], SC_sb[:], start=True, stop=True
                    )

                    # ---- sumsq via mbd-masked row reduce
                    junk = small.tile([P, 128], f32)
                    sumsq = small.tile([P, 1], f32)
                    nc.vector.scalar_tensor_tensor(
                        out=junk[:],
                        in0=Mout[0:P, 0:128],
                        scalar=1.0,
                        in1=mbdf[:],
                        op0=AluOpType.mult,
                        op1=AluOpType.mult,
                        accum_out=sumsq[:],
                    )
                    # rsq = exp(-0.5 * ln(sumsq/D + eps))
                    lnu = small.tile([P, 1], f32)
                    nc.scalar.activation(
                        lnu[:], sumsq[:], mybir.ActivationFunctionType.Ln,
                        bias=epsb[:], scale=1.0 / D,
                    )
                    rsq = small.tile([P, 1], f32)
                    nc.scalar.activation(
                        rsq[:], lnu[:], mybir.ActivationFunctionType.Exp,
                        scale=-0.5,
                    )
                    scoresR = small.tile([P, L], f32)
                    nc.scalar.activation(
                        scoresR[:], Mout[0:P, 128 : 128 + L],
                        mybir.ActivationFunctionType.Copy, scale=rsq[:],
                    )

                    # ---- masked softmax over sources (free axis)
                    scoreT = ps_sm.tile([L, P], f32, tag="sm")
                    nc.tensor.transpose(scoreT[:], scoresR[:], ident[:P, :P])
                    smask = small.tile([L, P], f32)
                    nc.vector.tensor_add(smask[:], scoreT[:], maskneg[:])
                    esc = small.tile([L, P], f32)
                    nc.scalar.activation(
                        esc[:], smask[:], mybir.ActivationFunctionType.Exp
                    )
                    ssum = small.tile([L, R], f32)
                    nc.vector.reduce_sum(
                        ssum[:],
                        esc.rearrange("p (r n) -> p r n", n=NS),
                        axis=mybir.AxisListType.X,
                    )
                    rec = small.tile([L, R], f32)
                    nc.vector.reciprocal(rec[:], ssum[:])
                    alpha = small.tile([L, P], f32)
                    nc.vector.tensor_tensor(
                        alpha.rearrange("p (r n) -> p r n", n=NS),
                        esc.rearrange("p (r n) -> p r n", n=NS),
                        rec.unsqueeze(2).broadcast_to([L, R, NS]),
                        AluOpType.mult,
                    )

                    # ---- fold alphas through M: BT = M_bd.T @ abd
                    alphaT = ps_sm.tile([P, L], f32, tag="sm")
                    nc.tensor.transpose(alphaT[:], alpha[:], ident[:L, :L])
                    abd = small.tile([128, 128], bf16, tag="abd")
                    if s_idx < 2:  # zero pads persist in the 2 buffers
                        nc.vector.memset(abd[:, 120:128], 0.0)
                        nc.vector.memset(abd[96:128], 0.0)
                    nc.vector.scalar_tensor_tensor(
                        out=abd[0:P, 0:120].rearrange("p (r l) -> p r l", l=L),
                        in0=alphaT.unsqueeze(1).broadcast_to([P, R, L]),
                        scalar=1.0,
                        in1=diagm.rearrange("p (r l) -> p r l", l=L),
                        op0=AluOpType.mult,
                        op1=AluOpType.mult,
                    )
                    BTp = ps_sm.tile([128, 128], f32, tag="sm")
                    nc.tensor.matmul(
                        BTp[:], mbd[:], abd[:], start=True, stop=True
                    )
                    btsb = small.tile([128, 128], bf16, tag="bt")
                    nc.scalar.copy(btsb[:], BTp[:])

                    # ---- H = BT.T @ X (bf16, N=512 per PSUM bank)
                    for nb in range(4):
                        Hp = ps_h.tile([128, 512], f32)
                        nc.tensor.matmul(
                            Hp[:],
                            btsb[:],
                            X4[:, s, 512 * nb : 512 * (nb + 1)],
                            start=True,
                            stop=True,
                        )
                        if nb % 2 == 0:
                            nc.scalar.copy(
                                h4[:, s, 512 * nb : 512 * (nb + 1)],
                                Hp[0 : R * L, :],
                            )
                        else:
                            nc.vector.tensor_copy(
                                h4[:, s, 512 * nb : 512 * (nb + 1)],
                                Hp[0 : R * L, :],
                            )
                    s_idx += 1

                # ---- one contiguous 1.92MB store per super (ACT HWDGE ring)
                nc.scalar.dma_start(
                    outT[sup_idx * R * L : (sup_idx + 1) * R * L, :]
                    .rearrange("p (s d) -> p s d", d=D),
                    h4[:],
                )

    real_gat = bacc.get_activation_tables
    AF = mybir.ActivationFunctionType

    def gat_pinned(arch):
        out = {}
        for name, fns in real_gat(arch).items():
            if name == "natural_log_exp_and_others":
                out[name] = set(fns)
            else:
                out[name] = {f for f in fns if f not in (AF.Ln, AF.Exp)}
        return out

    bacc.get_activation_tables = gat_pinned
    try:
        if do_compile:
            nc.compile()
    finally:
        bacc.get_activation_tables = real_gat
    return nc


_NC_CACHE = None


def _prep_loT(layer_outputs, embedding):
    """[L,B,T,D]+[B,T,D] -> p-major per-core per-super stacks
    [N_CORES, NSUP * 125, SB * D] bf16."""
    lo3 = np.empty((B * T, NJ, D), dtype=BF)
    lo3[:, 0, :] = embedding.reshape(B * T, D).astype(BF)
    lo3[:, 1:, :] = (
        layer_outputs.reshape(L, B * T, D).transpose(1, 0, 2).astype(BF)
    )
    sup = np.asarray(_super_starts())  # [NSUP]
    rows = sup[:, None] + np.arange(R * SB)[None, :]  # [NSUP, 20]
    out = np.empty((N_CORES, NSUP * P, SB * D), dtype=BF)
    for c in range(N_CORES):
        g = lo3[c * ROWS_PER_CORE + rows]          # [NSUP, 20, 25, D]
        g = g.reshape(NSUP, SB, P, D).transpose(0, 2, 1, 3)  # [NSUP,125,SB,D]
        out[c] = g.reshape(NSUP * P, SB * D)
    return out


def _make_in_maps(layer_outputs, embedding, queries, key_norm_weight):
    loT = _prep_loT(layer_outputs, embedding)
    consts = _build_consts(queries, key_norm_weight)
    in_maps = []
    for c in range(N_CORES):
        in_maps.append({
            "loT": loT[c],
            "qwT": consts["qwT"],
            "mtbd": consts["mtbd"],
            "mbd": consts["mbd"],
            "mbdf": consts["mbdf"],
            "diagm": consts["diagm"],
            "maskneg": consts["maskneg"],
            "ident": consts["ident"],
            "identb": consts["identb"],
        })
    return in_maps


def kernel(layer_outputs, embedding, queries, key_norm_weight):
    global _NC_CACHE
    layer_outputs = np.asarray(layer_outputs, dtype=np.float32)
    embedding = np.asarray(embedding, dtype=np.float32)
    queries = np.asarray(queries, dtype=np.float32)
    key_norm_weight = np.asarray(key_norm_weight, dtype=np.float32)

    in_maps = _make_in_maps(layer_outputs, embedding, queries, key_norm_weight)

    if _NC_CACHE is None:
        _NC_CACHE = build_kernel()
    nc = _NC_CACHE

    res = run_bass_kernel_spmd(nc, in_maps, core_ids=list(range(N_CORES)))

    sup = np.asarray(_super_starts())
    rows = (sup[:, None, None] + R * np.arange(SB)[None, :, None]
            + np.arange(R)[None, None, :])  # [NSUP, SB, R]
    full = np.empty((L, B * T, D), dtype=np.float32)
    for c in range(N_CORES):
        o = res.results[c]["outT"].astype(np.float32)
        o = o.reshape(NSUP, R * L, SB, D).transpose(0, 2, 1, 3)
        o = o.reshape(NSUP, SB, R, L, D).transpose(3, 0, 1, 2, 4)
        full[:, c * ROWS_PER_CORE + rows.ravel(), :] = o.reshape(
            L, NSUP * SB * R, D)
    return full.reshape(L, B, T, D)


# revision 32
# speedup vs baseline: 1.3476x; 1.0828x over previous
"""Trainium2 Bass kernel for BlockAttnRes.compute_all_inputs (bf16 pipeline).

Proven v2: 314880 ns, rel err 3.28e-03. Per-batch (5 rows) pipeline,
SWDGE loads, fold-transposes, bf16 I/O end-to-end.
"""

import numpy as np
import ml_dtypes

import concourse.bass as bass
import concourse.bacc as bacc
import concourse.mybir as mybir
from concourse import tile
from concourse.alu_op_type import AluOpType
from concourse.bass_utils import run_bass_kernel_spmd

L = 24
D = 2048
NUM_BLOCKS = 8
EPS = 1e-6
B, T = 2, 1024
N_CORES = 8

ROWS_PER_CORE = (B * T) // N_CORES  # 256
R = 5             # rows per batch
NJ = 25           # raw vectors per row: emb + 24 layer outputs
NS = 25           # sources per row
P = NJ * R        # 125 partitions per batch
NCHUNK = D // 128  # 16 d-chunks
CW = 152          # vt_sb per-chunk pitch: 128 (VT + 3 zero) + 24 qwT
NEG = -1e30

f32 = mybir.dt.float32
bf16 = mybir.dt.bfloat16
BF = ml_dtypes.bfloat16


def _source_matrix():
    M = np.zeros((NS, NJ), dtype=np.float32)
    M[0, 0] = 1.0
    for k in range(NUM_BLOCKS):
        for i in range(3):
            M[1 + 3 * k + i, 1 + 3 * k : 1 + 3 * k + i + 1] = 1.0
    return M


def _valid_matrix():
    V = np.zeros((L, NS), dtype=bool)
    for l in range(L):
        kb, ii = l // 3, l % 3
        V[l, 0] = True
        for k in range(kb):
            V[l, 3 * k + 3] = True
        if ii > 0:
            V[l, 3 * kb + ii] = True
    return V


def _build_consts(queries, key_norm_weight):
    M = _source_matrix()
    valid = _valid_matrix()
    eye_r = np.eye(R, dtype=np.float32)

    qw = (queries * key_norm_weight[None, :]).astype(np.float32)  # [L, D]
    qwT = np.ascontiguousarray(
        qw.reshape(L, NCHUNK, 128).transpose(2, 1, 0).reshape(128, NCHUNK * L)
    ).astype(BF)

    mtbd = np.einsum("nj,ab->ajbn", M, eye_r).reshape(P, NS * R)
    mtbd128 = np.zeros((P, 128), np.float32)
    mtbd128[:, :P] = mtbd
    mtbd128 = mtbd128.astype(BF)
    mbd = np.einsum("nj,ab->anbj", M, eye_r).reshape(NS * R, P)
    mbd128 = np.zeros((P, 128), np.float32)
    mbd128[:, :P] = mbd
    mbd128 = mbd128.astype(BF)
    eye_bd = np.zeros((P, 128), np.float32)
    eye_bd[:, :P] = np.eye(P, dtype=np.float32)
    diagm = np.einsum("ab,nl->anbl", eye_r, np.ones((NS, L), np.float32))
    diagm = np.ascontiguousarray(diagm.reshape(P, R * L)).astype(np.float32)
    maskneg = np.where(valid[:, None, :], 0.0, NEG)
    maskneg = np.broadcast_to(maskneg, (L, R, NS)).reshape(L, R * NS)
    maskneg = np.ascontiguousarray(maskneg).astype(np.float32)

    ident = np.eye(128, dtype=np.float32)
    return dict(qwT=qwT, mtbd=mtbd128, mbd=mbd128, eyebd=eye_bd, diagm=diagm,
                maskneg=maskneg, ident=ident)


def _batch_starts():
    starts = [R * b for b in range(ROWS_PER_CORE // R)]  # 0..250
    if starts[-1] + R < ROWS_PER_CORE:
        starts.append(ROWS_PER_CORE - R)  # 251 (overlaps; identical rewrites)
    return starts


def build_kernel(do_compile=True):
    nc = bacc.Bacc("TRN2", target_bir_lowering=False, debug=False)

    loT = nc.dram_tensor("loT", [ROWS_PER_CORE * NJ, D], bf16,
                         kind="ExternalInput").ap()
    qwT_d = nc.dram_tensor("qwT", [128, NCHUNK * L], bf16, kind="ExternalInput").ap()
    mtbd_d = nc.dram_tensor("mtbd", [P, 128], bf16, kind="ExternalInput").ap()
    mbd_d = nc.dram_tensor("mbd", [P, 128], bf16, kind="ExternalInput").ap()
    eyebd_d = nc.dram_tensor("eyebd", [P, 128], f32, kind="ExternalInput").ap()
    diagm_d = nc.dram_tensor("diagm", [P, R * L], f32, kind="ExternalInput").ap()
    maskneg_d = nc.dram_tensor("maskneg", [L, R * NS], f32, kind="ExternalInput").ap()
    ident_d = nc.dram_tensor("ident", [128, 128], f32, kind="ExternalInput").ap()
    outT = nc.dram_tensor("outT", [ROWS_PER_CORE * L, D], bf16,
                          kind="ExternalOutput").ap()

    with tile.TileContext(nc) as tc:
        with (
            tc.tile_pool(name="const", bufs=1) as const,
            tc.tile_pool(name="xpool", bufs=6) as xpool,
            tc.tile_pool(name="vtpool", bufs=3) as vtpool,
            tc.tile_pool(name="hpool", bufs=3) as hpool,
            tc.tile_pool(name="small", bufs=2) as small,
            tc.tile_pool(name="ps_ft", bufs=2, space=bass.MemorySpace.PSUM) as ps_ft,
            tc.tile_pool(name="ps_sc", bufs=2, space=bass.MemorySpace.PSUM) as ps_sc,
            tc.tile_pool(name="ps_sm", bufs=2, space=bass.MemorySpace.PSUM) as ps_sm,
            tc.tile_pool(name="ps_h", bufs=2, space=bass.MemorySpace.PSUM) as ps_h,
        ):
            qwT = const.tile([128, NCHUNK * L], bf16)
            nc.sync.dma_start(qwT[:], qwT_d[:])
            mtbd = const.tile([P, 128], bf16)
            nc.sync.dma_start(mtbd[:], mtbd_d[:])
            mbd = const.tile([P, 128], bf16)
            nc.sync.dma_start(mbd[:], mbd_d[:])
            eyebd = const.tile([P, 128], f32)
            nc.sync.dma_start(eyebd[:], eyebd_d[:])
            diagm = const.tile([P, R * L], f32)
            nc.sync.dma_start(diagm[:], diagm_d[:])
            maskneg = const.tile([L, R * NS], f32)
            nc.sync.dma_start(maskneg[:], maskneg_d[:])
            ident = const.tile([128, 128], f32)
            nc.sync.dma_start(ident[:], ident_d[:])
            epsb = const.tile([P, 1], f32)
            nc.vector.memset(epsb[:], EPS)

            for row0 in _batch_starts():
                X = xpool.tile([P, D], bf16)
                nc.gpsimd.dma_start(X[:], loT[row0 * NJ : row0 * NJ + P, :])

                vt_sb = vtpool.tile([128, NCHUNK * CW], bf16)
                vt3 = vt_sb.rearrange("p (c w) -> p c w", w=CW)
                nc.scalar.copy(
                    vt3[:, :, 128 : 128 + L],
                    qwT.rearrange("p (c w) -> p c w", w=L),
                )
                for half in range(4):
                    ftp = ps_ft.tile([128, 512], f32)
                    for cc in range(4):
                        c = 4 * half + cc
                        nc.tensor.matmul(
                            ftp[:, 128 * cc : 128 * (cc + 1)],
                            X[:, 128 * c : 128 * (c + 1)],
                            mtbd[:],
                            start=True,
                            stop=True,
                        )
                    ft4 = ftp.rearrange("p (cc w) -> p cc w", w=128)
                    dst = vt3[:, 4 * half : 4 * half + 4, 0:128]
                    if half % 2 == 0:
                        nc.scalar.copy(dst, ft4)
                    else:
                        nc.vector.tensor_copy(dst, ft4)

                SCp = ps_sc.tile([128, CW], f32)
                for c in range(NCHUNK):
                    nc.tensor.matmul(
                        SCp[:],
                        vt3[:, c, 0:128],
                        vt3[:, c, 0:CW],
                        start=(c == 0),
                        stop=(c == NCHUNK - 1),
                    )

                junk = small.tile([P, 128], f32)
                sumsq = small.tile([P, 1], f32)
                nc.vector.scalar_tensor_tensor(
                    out=junk[:],
                    in0=SCp[0:P, 0:128],
                    scalar=1.0,
                    in1=eyebd[:],
                    op0=AluOpType.mult,
                    op1=AluOpType.mult,
                    accum_out=sumsq[:],
                )
                lnu = small.tile([P, 1], f32)
                nc.scalar.activation(
                    lnu[:], sumsq[:], mybir.ActivationFunctionType.Ln,
                    bias=epsb[:], scale=1.0 / D,
                )
                rsq = small.tile([P, 1], f32)
                nc.scalar.activation(
                    rsq[:], lnu[:], mybir.ActivationFunctionType.Exp, scale=-0.5
                )
                scoresR = small.tile([P, L], f32)
                nc.scalar.activation(
                    scoresR[:], SCp[0:P, 128 : 128 + L],
                    mybir.ActivationFunctionType.Copy, scale=rsq[:],
                )

                scoreT = ps_sm.tile([L, P], f32, tag="sm")
                nc.tensor.transpose(scoreT[:], scoresR[:], ident[:P, :P])
                smask = small.tile([L, P], f32)
                nc.vector.tensor_add(smask[:], scoreT[:], maskneg[:])
                esc = small.tile([L, P], f32)
                nc.scalar.activation(
                    esc[:], smask[:], mybir.ActivationFunctionType.Exp
                )
                ssum = small.tile([L, R], f32)
                nc.vector.reduce_sum(
                    ssum[:],
                    esc.rearrange("p (r n) -> p r n", r=R),
                    axis=mybir.AxisListType.X,
                )
                rec = small.tile([L, R], f32)
                nc.vector.reciprocal(rec[:], ssum[:])
                alpha = small.tile([L, P], f32)
                nc.vector.tensor_tensor(
                    alpha.rearrange("p (r n) -> p r n", r=R),
                    esc.rearrange("p (r n) -> p r n", r=R),
                    rec.unsqueeze(2).broadcast_to([L, R, NS]),
                    AluOpType.mult,
                )

                alphaT = ps_sm.tile([P, L], f32, tag="sm")
                nc.tensor.transpose(alphaT[:], alpha[:], ident[:L, :L])
                abd = small.tile([P, 128], bf16)
                nc.vector.memset(abd[:, 120:128], 0.0)
                nc.vector.scalar_tensor_tensor(
                    out=abd[:, 0:120].rearrange("p (r l) -> p r l", r=R),
                    in0=alphaT.unsqueeze(1).broadcast_to([P, R, L]),
                    scalar=1.0,
                    in1=diagm.rearrange("p (r l) -> p r l", r=R),
                    op0=AluOpType.mult,
                    op1=AluOpType.mult,
                )
                BTp = ps_sm.tile([128, 128], f32, tag="sm")
                nc.tensor.matmul(BTp[:], mbd[:], abd[:], start=True, stop=True)
                btsb = small.tile([128, 128], bf16)
                nc.scalar.copy(btsb[:], BTp[:])

                h_sb = hpool.tile([R * L, D], bf16)
                for nb in range(4):
                    Hp = ps_h.tile([128, 512], f32)
                    nc.tensor.matmul(
                        Hp[:],
                        btsb[0:P, :],
                        X[:, 512 * nb : 512 * (nb + 1)],
                        start=True,
                        stop=True,
                    )
                    if nb % 2 == 0:
                        nc.scalar.copy(h_sb[:, 512 * nb : 512 * (nb + 1)],
                                       Hp[0 : R * L, :])
                    else:
                        nc.vector.tensor_copy(h_sb[:, 512 * nb : 512 * (nb + 1)],
                                              Hp[0 : R * L, :])

                nc.scalar.dma_start(
                    outT[row0 * L : row0 * L + R * L, :], h_sb[:]
                )

    real_gat = bacc.get_activation_tables
    AF = mybir.ActivationFunctionType

    def gat_pinned(arch):
        out = {}
        for name, fns in real_gat(arch).items():
            if name == "natural_log_exp_and_others":
                out[name] = set(fns)
            else:
                out[name] = {f for f in fns if f not in (AF.Ln, AF.Exp)}
        return out

    bacc.get_activation_tables = gat_pinned
    try:
        if do_compile:
            nc.compile()
    finally:
        bacc.get_activation_tables = real_gat
    return nc


_NC_CACHE = None


def _prep_loT(layer_outputs, embedding):
    loT = np.empty((B * T, NJ, D), dtype=BF)
    loT[:, 0, :] = embedding.reshape(B * T, D).astype(BF)
    loT[:, 1:, :] = (
        layer_outputs.reshape(L, B * T, D).transpose(1, 0, 2).astype(BF)
    )
    return loT


def _make_in_maps(layer_outputs, embedding, queries, key_norm_weight):
    loT = _prep_loT(layer_outputs, embedding)
    consts = _build_consts(queries, key_norm_weight)
    in_maps = []
    for c in range(N_CORES):
        r0 = c * ROWS_PER_CORE
        in_maps.append({
            "loT": loT[r0 : r0 + ROWS_PER_CORE].reshape(ROWS_PER_CORE * NJ, D),
            "qwT": consts["qwT"],
            "mtbd": consts["mtbd"],
            "mbd": consts["mbd"],
            "eyebd": consts["eyebd"],
            "diagm": consts["diagm"],
            "maskneg": consts["maskneg"],
            "ident": consts["ident"],
        })
    return in_maps


def kernel(layer_outputs, embedding, queries, key_norm_weight):
    global _NC_CACHE
    layer_outputs = np.asarray(layer_outputs, dtype=np.float32)
    embedding = np.asarray(embedding, dtype=np.float32)
    queries = np.asarray(queries, dtype=np.float32)
    key_norm_weight = np.asarray(key_norm_weight, dtype=np.float32)

    in_maps = _make_in_maps(layer_outputs, embedding, queries, key_norm_weight)

    if _NC_CACHE is None:
        _NC_CACHE = build_kernel()
    nc = _NC_CACHE

    res = run_bass_kernel_spmd(nc, in_maps, core_ids=list(range(N_CORES)))

    full = np.empty((L, B * T, D), dtype=np.float32)
    for c in range(N_CORES):
        r0 = c * ROWS_PER_CORE
        outT = res.results[c]["outT"].astype(np.float32).reshape(
            ROWS_PER_CORE, L, D
        )
        full[:, r0 : r0 + ROWS_PER_CORE, :] = outT.transpose(1, 0, 2)
    return full.reshape(L, B, T, D)
